# revision 35
# baseline (speedup 1.0000x reference)
"""Trainium2 Bass kernel for nn_CryptoGNN (2-layer GCN + pooled heads).

Math (same collapse as the validated baseline):
  With A = normalized adjacency (incl. self loops), P = [B,N] pooling,
  u[d] = sum_{s->d} dis[s]x[s],   zhat = (u + dis*x_self)@W1 + sqrt(deg)*b1,
  h1hat = relu(zhat);  true h1 = dis*h1hat, so the pooling matrix columns
  are pre-scaled by dis and layer 2 + heads collapse to tiny host math.

Per-core device pipeline (8-way node sharding, 12544 dst nodes/core),
node-PAIRED layout: dst nodes 2m/2m+1 share one stream segment per bank
group -- even nodes' features ride partition rows 16g+0..5, odd nodes'
rows 16g+6..11, sqrt(deg) lanes rows 16g+12/13, with the shorter side
zero-padded so the pair shares ONE segment end position.  The self loop
is injected as an extra stream edge (in the bank with most slack), so
no separate aug matmul is needed.
  Per dst-chunk c (host packs [xs fp8 | mask fp8 | bidx i16] per chunk):
    1. one DMA for the chunk's packed region
    2. DVE segmented scan: state = mask*state + value (fp32 state)
    3. GPSIMD ap_gather at the nd/2 host-known pair-end positions ->
       [128, nd/2] with even sums on even lanes, odd sums on odd lanes
    4. DVE tensor_copy fp32->bf16 into dt (HALF width: columns = pairs)
    5. per pair-tile: z_e = dt_t^T @ selw_e, z_o = dt_t^T @ selw_o
       (selw_e routes even lanes + b1 on the sqrt lane; selw_o the odd
       lanes), relu -> h1 fp8 (Act/DVE), G^T += h1^T @ papt (fp8, host
       swizzled per parity) into one [128,80] PSUM over all 98 matmuls.
Host sums the 8 partial G^T and runs the small head in numpy.
"""

import sys

if "/opt/trn_rl_repo" not in sys.path:
    sys.path.insert(0, "/opt/trn_rl_repo")

import numpy as np
import ml_dtypes

N = 100000
E = 600000
B = 64
IN = 6
H = 128
S = 16

NG = 8                    # banks and cores
NS = 12544                # nodes per core shard (98*128)
NP2 = NS // 2             # node pairs per core
NPAD = NS * NG            # 100352
NT = 98                   # node tiles per shard
NTP = 49                  # pair tiles per shard
# dst chunks per core, in node tiles (all even); small ends, big middle
TCH = (2, 4, 8, 12, 12, 16, 16, 16, 6, 4, 2)
C = len(TCH)
NDCS = tuple(t * 128 for t in TCH)          # nodes per chunk
NPCS = tuple(n // 2 for n in NDCS)          # pairs per chunk
DOFF = tuple(int(x) for x in np.concatenate([[0], np.cumsum(NDCS)]))
POFF = tuple(d // 2 for d in DOFF)
PCOL = 80                 # papt columns: 64 PA + <=16 local P
P128 = 128

_compiled = {}


def _region_layout(JWS):
    """Per-chunk packed byte region: [xs L | mask L | bidx ndp/8 | pad]."""
    RO = [0]
    for c in range(C):
        w = int(JWS[c])
        width = 2 * w + NPCS[c] // 8
        width = (width + 31) & ~31
        RO.append(RO[-1] + width)
    return RO


def _build_nc(JWS):
    import concourse.bacc as bacc
    import concourse.mybir as mybir
    from concourse import tile

    f32 = mybir.dt.float32
    bf16 = mybir.dt.bfloat16
    fp8 = mybir.dt.float8e4
    i16 = mybir.dt.int16

    RO = _region_layout(JWS)
    XMW = RO[-1]
    JWMAX = max(int(w) for w in JWS)
    NBMAX = max(NPCS)

    nc = bacc.Bacc("TRN2", target_bir_lowering=False, debug=False)

    xm = nc.declare_dram_parameter("xm", [P128, XMW], fp8, isOutput=False)
    selw = nc.declare_dram_parameter("selw", [P128, 2 * H], bf16, isOutput=False)
    papt = nc.declare_dram_parameter("papt", [P128, NTP * 2 * PCOL], fp8,
                                     isOutput=False)
    gout = nc.declare_dram_parameter("gout", [P128, PCOL], f32, isOutput=True)

    with tile.TileContext(nc) as tc:
        with (
            tc.tile_pool(name="big", bufs=1) as big,
            tc.tile_pool(name="small", bufs=1) as small,
            tc.tile_pool(name="scp", bufs=3) as scp,
            tc.tile_pool(name="d32p", bufs=2) as d32p,
            tc.tile_pool(name="hbuf", bufs=6) as hbuf,
            tc.tile_pool(name="psz", bufs=3, space="PSUM") as pszp,
            tc.tile_pool(name="psG", bufs=1, space="PSUM") as psGp,
        ):
            # preload the activation-function table while DMAs run
            warm = small.tile([1, 2], f32)
            nc.vector.memset(warm[:], 0.0)
            nc.scalar.activation(out=warm[:], in_=warm[:],
                                 func=mybir.ActivationFunctionType.Copy)

            xm_t = big.tile([P128, XMW], fp8, tag="xmb")
            dt = big.tile([P128, NP2], bf16, tag="dt")
            papt_t = big.tile([P128, NTP * 2 * PCOL], fp8, tag="papt")

            selw_t = small.tile([P128, H], bf16, name="selw_e")
            selwo_t = small.tile([P128, H], bf16, name="selw_o")

            scs = [None] * C
            d32s = [None] * C

            def dma_xm(c0, c1):
                nc.sync.dma_start(out=xm_t[:, RO[c0]:RO[c1]],
                                  in_=xm[:, RO[c0]:RO[c1]])

            def dma_papt(t0, t1):
                p0, p1 = t0 * 2 * PCOL, t1 * 2 * PCOL
                nc.sync.dma_start(out=papt_t[:, p0:p1], in_=papt[:, p0:p1])

            def scan_c(c):
                o = RO[c]
                w = int(JWS[c])
                sc = scp.tile([P128, JWMAX], f32, tag=f"sc{c % 3}",
                              name=f"scan_{c}")
                nc.vector.tensor_tensor_scan(
                    out=sc[:, 0:w],
                    data0=xm_t[:, o + w:o + 2 * w],
                    data1=xm_t[:, o:o + w],
                    initial=0.0, op0=mybir.AluOpType.mult,
                    op1=mybir.AluOpType.add,
                )
                scs[c] = sc

            def g2_c(c):
                ndp = NPCS[c]
                o = RO[c] + 2 * int(JWS[c])
                d32 = d32p.tile([P128, NBMAX], f32, tag=f"d32{c % 2}",
                                name=f"d32_{c}")
                nc.gpsimd.ap_gather(
                    out_ap=d32[:, 0:ndp], in_ap=scs[c][:, 0:int(JWS[c])],
                    idxs_ap=xm_t[:, o:o + ndp // 8].bitcast(i16),
                    channels=P128, num_elems=int(JWS[c]), d=1, num_idxs=ndp,
                )
                d32s[c] = d32

            def cvt_c(c):
                p0, ndp = POFF[c], NPCS[c]
                if c <= 4:
                    # early chunks: Act is idle while DVE is scan-bound
                    nc.scalar.activation(
                        out=dt[:, p0:p0 + ndp], in_=d32s[c][:, 0:ndp],
                        func=mybir.ActivationFunctionType.Copy,
                    )
                else:
                    nc.vector.tensor_copy(out=dt[:, p0:p0 + ndp],
                                          in_=d32s[c][:, 0:ndp])

            # ---------- issue order ----------
            dma_xm(0, 1)
            dma_xm(1, 2)
            dma_xm(2, 3)
            dma_xm(3, 4)
            nc.sync.dma_start(out=selw_t[:], in_=selw[:, 0:H])
            nc.sync.dma_start(out=selwo_t[:], in_=selw[:, H:2 * H])
            dma_xm(4, 5)
            dma_xm(5, 6)
            dma_xm(6, 7)
            dma_papt(0, 24)
            dma_xm(7, 8)
            dma_xm(8, 9)
            dma_papt(24, NTP)
            dma_xm(9, C)

            # ---------- fused pipeline: each chunk's z/relu/G batches are
            # emitted right after its cvt, so every engine's in-order queue
            # matches data-readiness order ----------
            G_ps = psGp.tile([P128, PCOL], f32, tag="G")
            QB = 4                     # pair-tiles per batch (2 mms each)
            batches_of = {}
            for c in range(C):
                t = POFF[c] // 128
                left = TCH[c] // 2
                bl = []
                while left > 0:
                    sz = min(QB, left)
                    bl.append((t, sz))
                    t += sz
                    left -= sz
                batches_of[c] = bl

            def z_mms(t0, m, ps):
                for u in range(m):
                    p0 = (t0 + u) * P128
                    for par, rw in ((0, selw_t), (1, selwo_t)):
                        nc.tensor.matmul(
                            out=ps[:, (2 * u + par) * H:(2 * u + par + 1) * H],
                            lhsT=dt[:, p0:p0 + P128],
                            rhs=rw[:],
                            start=True, stop=True,
                        )

            def g_mms(t0, m, h1):
                for u in range(m):
                    t = t0 + u
                    for par in (0, 1):
                        nc.tensor.matmul(
                            out=G_ps[:],
                            lhsT=h1[:, (2 * u + par) * H:(2 * u + par + 1) * H],
                            rhs=papt_t[:, (2 * t + par) * PCOL:
                                       (2 * t + par + 1) * PCOL],
                            start=(t == 0 and par == 0),
                            stop=(t == NTP - 1 and par == 1),
                        )

            prev = [None]

            def emit_batches(c):
                for (t0, m) in batches_of[c]:
                    ps = pszp.tile([P128, QB * 2 * H], f32, tag="z")
                    z_mms(t0, m, ps)
                    h1 = hbuf.tile([P128, QB * 2 * H], fp8, tag="h1")
                    if t0 >= POFF[6] // 128:
                        # phase A is done by these late batches; split their
                        # relus across Act and DVE to shorten the drain
                        hm = m * H
                        nc.scalar.activation(
                            out=h1[:, :hm], in_=ps[:, :hm],
                            func=mybir.ActivationFunctionType.Relu,
                        )
                        nc.vector.tensor_scalar_max(
                            out=h1[:, hm:2 * m * H], in0=ps[:, hm:2 * m * H],
                            scalar1=0.0,
                        )
                    else:
                        nc.scalar.activation(
                            out=h1[:, :2 * m * H], in_=ps[:, :2 * m * H],
                            func=mybir.ActivationFunctionType.Relu,
                        )
                    if prev[0] is not None:
                        g_mms(*prev[0])
                    prev[0] = (t0, m, h1)

            scan_c(0)
            scan_c(1)
            g2_c(0)
            for c in range(2, C):
                scan_c(c)
                g2_c(c - 1)
                cvt_c(c - 2)
                if c >= 3:
                    emit_batches(c - 3)
            g2_c(C - 1)
            cvt_c(C - 2)
            emit_batches(C - 3)
            cvt_c(C - 1)
            emit_batches(C - 2)
            emit_batches(C - 1)
            g_mms(*prev[0])

            G_sb = small.tile([P128, PCOL], f32)
            nc.vector.tensor_copy(out=G_sb[:], in_=G_ps[:])
            nc.sync.dma_start(out=gout[:], in_=G_sb[:])

    nc.compile()
    return nc


def _preprocess(x, edge_index, batch_idx):
    """Integer/structure preprocessing -> per-core device inputs."""
    src = np.asarray(edge_index[0], dtype=np.int64)
    dst = np.asarray(edge_index[1], dtype=np.int64)

    deg = (np.bincount(dst, minlength=N) + 1).astype(np.float32)
    dis = (1.0 / np.sqrt(deg)).astype(np.float32)
    sq = np.sqrt(deg).astype(np.float32)
    dis_pad = np.zeros(NPAD, np.float32)
    dis_pad[:N] = dis
    sq_pad = np.zeros(NPAD, np.float32)
    sq_pad[:N] = sq

    bi = np.asarray(batch_idx, dtype=np.int64)
    cnt = np.bincount(bi, minlength=B).astype(np.float32)

    x_np = np.asarray(x, dtype=np.float32)
    x_pad = np.zeros((NPAD, IN), np.float32)
    x_pad[:N] = x_np
    disx = x_pad * dis_pad[:, None]          # [NPAD, 6]

    # ---- pooling matrices (dense PA = P @ A) ----
    loop = np.arange(N, dtype=np.int64)
    src2 = np.concatenate([src, loop])
    dst2 = np.concatenate([dst, loop])
    w = (dis[src2] * dis[dst2]).astype(np.float64)
    flat = bi[dst2] * NPAD + src2
    PA = np.bincount(flat, weights=w, minlength=B * NPAD).reshape(B, NPAD)
    PA = PA.astype(np.float32)
    Pm = np.zeros((B, NPAD), np.float32)
    Pm[bi, np.arange(N)] = 1.0
    papt_full = (np.concatenate([PA, Pm], axis=0) * dis_pad[None, :]).T  # [NPAD,128]

    # graph span per core (for the P columns)
    first_graph = np.zeros(NG, np.int64)
    span = np.zeros(NG, np.int64)
    for k in range(NG):
        lo, hi = k * NS, min((k + 1) * NS, N)
        if lo >= N:
            first_graph[k] = B - 1
            span[k] = 1
            continue
        gset = bi[lo:hi]
        first_graph[k] = gset[0]
        span[k] = gset[-1] - gset[0] + 1
        assert span[k] <= PCOL - B, f"graph span {span[k]} > {PCOL - B}"

    # ---- per-core edge banking ----
    core = dst // NS
    dst_local = dst - core * NS
    # bank = rank within (core, dst_local) group mod NG (balances each
    # node's edges across banks)
    key0 = core * NS + dst_local
    order0 = np.argsort(key0, kind="stable")
    grp = key0[order0]
    starts = np.r_[True, grp[1:] != grp[:-1]]
    run_id = np.cumsum(starts) - 1
    run_first = np.where(starts)[0]
    rank = np.arange(E) - run_first[run_id]
    bank_e = np.empty(E, np.int64)
    # rotate the bank start per node PAIR: both nodes of a pair share the
    # rotation so their per-bank segments overlap (s = max of the two),
    # and rotations balance the banks globally
    bank_e[order0] = (rank + (dst_local[order0] // 2)) % NG

    f8 = ml_dtypes.float8_e4m3
    disx_f8 = disx.astype(f8)
    sq_f8 = sq_pad.astype(f8)

    # per-(node,bank) edge counts; +1 self edge in the slackest bank
    le = np.zeros((NPAD, NG), np.int32)
    np.add.at(le, (dst, bank_e), 1)

    JWS = [0] * C
    per_core = []
    for k in range(NG):
        n0 = k * NS
        lek = le[n0:n0 + NS].copy()            # [NS, NG]
        pv = lek.reshape(NP2, 2, NG)
        slack = pv[:, ::-1, :] - pv            # partner_count - own
        # rotate tie-breaking per node so forced +1s spread across banks
        rot = (np.arange(NG)[None, :] + np.arange(NS)[:, None]) % NG
        score = slack.reshape(NS, NG).astype(np.int64) * 16 + rot
        gstar = np.argmax(score, axis=1)
        realn = (np.arange(n0, n0 + NS) < N)
        nn = np.arange(NS)[realn]
        lek[nn, gstar[realn]] += 1
        s = np.maximum(lek.reshape(NP2, 2, NG)[:, 0, :],
                       lek.reshape(NP2, 2, NG)[:, 1, :])   # [NP2, NG]
        per_core.append((lek, gstar, s))
        for c in range(C):
            m0, m1 = POFF[c], POFF[c + 1]
            ln = 1 + s[m0:m1].sum(axis=0)
            JWS[c] = max(JWS[c], int(ln.max()))
    JWS = [((wv + 31) // 32) * 32 for wv in JWS]
    RO = _region_layout(JWS)
    XMW = RO[-1]

    # edge slot ranks: sort edges by (core, bank, dst_local)
    keyE = (core * NG + bank_e) * NS + dst_local
    orderE = np.argsort(keyE, kind="stable")
    src_s = src[orderE]
    core_s = core[orderE]
    bank_s = bank_e[orderE]
    dstl_s = dst_local[orderE]
    grpE = keyE[orderE]
    startsE = np.r_[True, grpE[1:] != grpE[:-1]]
    runE = np.cumsum(startsE) - 1
    runE_first = np.where(startsE)[0]
    rankE = np.arange(E) - runE_first[runE]

    xm_all = np.zeros((NG, P128, XMW), f8)
    feat6 = np.arange(6)
    for k in range(NG):
        lek, gstar, s = per_core[k]
        n0 = k * NS
        ek = core_s == k
        e_src = src_s[ek]
        e_bank = bank_s[ek]
        e_dstl = dstl_s[ek]
        e_rank = rankE[ek]
        e_chunk = np.searchsorted(np.asarray(DOFF[1:]), e_dstl, side="right")
        for c in range(C):
            wv = int(JWS[c])
            ndp = NPCS[c]
            m0, m1 = POFF[c], POFF[c + 1]
            b0 = RO[c]
            sc = s[m0:m1]                       # [ndp, NG]
            # pair start positions per bank (lead slot at 0)
            pos = np.zeros((ndp, NG), np.int64)
            np.cumsum(sc[:-1], axis=0, out=pos[1:])
            pos += 1
            ends = pos + sc - 1                 # [ndp, NG]

            xs_c = np.zeros((P128, wv), f8)
            mk_c = np.ones((P128, wv), f8)
            for g in range(NG):
                nzm = sc[:, g] > 0
                cols = pos[nzm, g]
                mk_c[16 * g:16 * g + 14, cols] = 0.0
                mk_c[16 * g:16 * (g + 1), 0] = 0.0

            # edge values
            ec = e_chunk == c
            if ec.any():
                g_e = e_bank[ec]
                pr = e_dstl[ec] // 2 - m0
                par = e_dstl[ec] & 1
                slot = pos[pr, g_e] + e_rank[ec]
                rows6 = (16 * g_e + 6 * par)[:, None] + feat6[None, :]
                vals = disx_f8[e_src[ec]]       # [ne, 6]
                xs_c[rows6.ravel(), np.repeat(slot, 6)] = vals.ravel()

            # self edges: node n (real) in bank gstar[n] at its last own slot
            nloc = np.arange(m0 * 2, m1 * 2)
            realn = (n0 + nloc) < N
            nl = nloc[realn]
            if len(nl):
                gsn = gstar[nl]
                pr = nl // 2 - m0
                par = nl & 1
                own_len = lek[nl, gsn]
                slot = pos[pr, gsn] + own_len - 1
                rows6 = (16 * gsn + 6 * par)[:, None] + feat6[None, :]
                vals = disx_f8[n0 + nl]
                xs_c[rows6.ravel(), np.repeat(slot, 6)] = vals.ravel()
                # sqrt(deg) lanes at the pair start of the self bank
                srow = 16 * gsn + 12 + par
                spos = pos[pr, gsn]
                xs_c[srow, spos] = sq_f8[n0 + nl]

            # pair end positions -> bidx (0 for fully-empty pairs)
            bx_c = np.zeros((P128, ndp // 16), np.int16)
            for g in range(NG):
                bvals = np.where(sc[:, g] > 0, ends[:, g], 0)
                bx_c[16 * g:16 * (g + 1)] = (
                    bvals.reshape(ndp // 16, 16).T.astype(np.int16)
                )

            xm_all[k, :, b0:b0 + wv] = xs_c
            xm_all[k, :, b0 + wv:b0 + 2 * wv] = mk_c
            xm_all[k, :, b0 + 2 * wv:b0 + 2 * wv + ndp // 8] = bx_c.view(f8)

    # papt per core, parity-interleaved per pair-tile, device layout
    papt_all = np.zeros((NG, P128, NTP * 2 * PCOL), f8)
    for k in range(NG):
        n0 = k * NS
        pk = np.zeros((NS, PCOL), np.float32)
        pk[:, :B] = papt_full[n0:n0 + NS, :B]
        b0, sp = first_graph[k], span[k]
        pk[:, B:B + sp] = papt_full[n0:n0 + NS, B + b0:B + b0 + sp]
        # node 2(t*128+p)+par -> papt_all[p, (2t+par)*PCOL + j]
        pk4 = pk.reshape(NTP, P128, 2, PCOL)        # [t, p, par, j]
        papt_all[k] = (
            pk4.transpose(1, 0, 2, 3).reshape(P128, NTP * 2 * PCOL).astype(f8)
        )

    return {
        "JW": tuple(JWS),
        "JWS": JWS,
        "xm_all": xm_all,
        "papt_all": papt_all,
        "first_graph": first_graph,
        "span": span,
        "cnt": cnt,
    }


def _head(G, cnt, inputs):
    f = np.float32
    W2 = np.asarray(inputs["W2"], f)
    b2 = np.asarray(inputs["b2"], f)
    Wg = np.asarray(inputs["Wg"], f)
    bg = np.asarray(inputs["bg"], f)
    Et = np.asarray(inputs["Et"], f)
    Ek = np.asarray(inputs["Ek"], f)
    Ev = np.asarray(inputs["Ev"], f)
    Wp = np.asarray(inputs["Wp"], f)
    bp = np.asarray(inputs["bp"], f)
    Ekid = np.asarray(inputs["Ekid"], f)
    Wc = np.asarray(inputs["Wc"], f)
    bc = np.asarray(inputs["bc"], f)
    Wl = np.asarray(inputs["Wl"], f)
    bl = np.asarray(inputs["bl"], f)
    Wm1 = np.asarray(inputs["Wm1"], f)
    bm1 = np.asarray(inputs["bm1"], f)
    Wm2 = np.asarray(inputs["Wm2"], f)
    bm2 = np.asarray(inputs["bm2"], f)
    st = np.asarray(inputs["sol_type_idx"], np.int64)
    sk = np.asarray(inputs["sol_key_idx"], np.int64)
    sv = np.asarray(inputs["sol_val_idx"], np.int64)
    kid = np.asarray(inputs["kernel_id"], np.int64)
    cond = np.asarray(inputs["cond_vec"], f)
    loc = np.asarray(inputs["local_feats"], f)

    relu = lambda a: np.maximum(a, 0.0).astype(f)

    Ph2 = G[:B] @ W2 + cnt[:, None] * b2[None, :] + G[B:]
    g = (Ph2 / np.maximum(cnt, 1.0)[:, None]) @ Wg + bg

    seq_mean = np.concatenate(
        [Et[st].mean(axis=1), Ek[sk].mean(axis=1), Ev[sv].mean(axis=1)], axis=-1
    ).astype(f)
    p = relu(seq_mean @ Wp + bp)
    kvec = Ekid[kid]
    c = relu(cond @ Wc + bc)
    l = relu(loc @ Wl + bl)
    xf = np.concatenate([g, p, kvec, c, l], axis=1).astype(f)
    return (relu(xf @ Wm1 + bm1) @ Wm2 + bm2).astype(f)


def kernel(**inputs) -> np.ndarray:
    from concourse.bass_utils import run_bass_kernel_spmd

    pre = _preprocess(inputs["x"], inputs["edge_index"], inputs["batch_idx"])
    sig = pre["JW"]
    if sig not in _compiled:
        _compiled[sig] = _build_nc(tuple(pre["JWS"]))
    nc = _compiled[sig]

    W1 = np.asarray(inputs["W1"], np.float32)
    b1 = np.asarray(inputs["b1"], np.float32)
    W1b = W1.astype(ml_dtypes.bfloat16)
    b1b = b1.astype(ml_dtypes.bfloat16)
    selw = np.zeros((P128, 2 * H), ml_dtypes.bfloat16)
    for g in range(NG):
        selw[16 * g:16 * g + 6, 0:H] = W1b            # even lanes -> z_e
        selw[16 * g + 6:16 * g + 12, H:2 * H] = W1b   # odd lanes -> z_o
        selw[16 * g + 12, 0:H] = b1b                  # even sqrt(deg) lane
        selw[16 * g + 13, H:2 * H] = b1b              # odd sqrt(deg) lane

    in_maps = []
    for k in range(NG):
        in_maps.append({
            "xm": pre["xm_all"][k],
            "selw": selw,
            "papt": pre["papt_all"][k],
        })

    res = run_bass_kernel_spmd(nc, in_maps, core_ids=list(range(NG)))

    Gpa = np.zeros((B, H), np.float64)
    Gp = np.zeros((B, H), np.float64)
    for k, r in enumerate(res.results):
        gt = r["gout"].astype(np.float64)      # [128 f, 80 c]
        Gpa += gt[:, :B].T
        b0, sp = pre["first_graph"][k], pre["span"][k]
        Gp[b0:b0 + sp] += gt[:, B:B + sp].T
    G = np.concatenate([Gpa, Gp], axis=0).astype(np.float32)   # [128, H]

    return _head(G, pre["cnt"], inputs)


# revision 36
# speedup vs baseline: 1.0044x; 1.0044x over previous
"""Trainium2 Bass kernel for nn_CryptoGNN (2-layer GCN + pooled heads).

Math (same collapse as the validated baseline):
  With A = normalized adjacency (incl. self loops), P = [B,N] pooling,
  u[d] = sum_{s->d} dis[s]x[s],   zhat = (u + dis*x_self)@W1 + sqrt(deg)*b1,
  h1hat = relu(zhat);  true h1 = dis*h1hat, so the pooling matrix columns
  are pre-scaled by dis and layer 2 + heads collapse to tiny host math.

Per-core device pipeline (8-way node sharding, 12544 dst nodes/core),
node-PAIRED layout: dst nodes 2m/2m+1 share one stream segment per bank
group -- even nodes' features ride partition rows 16g+0..5, odd nodes'
rows 16g+6..11, sqrt(deg) lanes rows 16g+12/13, with the shorter side
zero-padded so the pair shares ONE segment end position.  The self loop
is injected as an extra stream edge (in the bank with most slack), so
no separate aug matmul is needed.
  Per dst-chunk c (host packs [xs fp8 | mask fp8 | bidx i16] per chunk):
    1. one DMA for the chunk's packed region
    2. DVE segmented scan: state = mask*state + value (fp32 state)
    3. GPSIMD ap_gather at the nd/2 host-known pair-end positions ->
       [128, nd/2] with even sums on even lanes, odd sums on odd lanes
    4. DVE tensor_copy fp32->bf16 into dt (HALF width: columns = pairs)
    5. per pair-tile: z_e = dt_t^T @ selw_e, z_o = dt_t^T @ selw_o
       (selw_e routes even lanes + b1 on the sqrt lane; selw_o the odd
       lanes), relu -> h1 fp8 (Act/DVE), G^T += h1^T @ papt (fp8, host
       swizzled per parity) into one [128,80] PSUM over all 98 matmuls.
Host sums the 8 partial G^T and runs the small head in numpy.
"""

import sys

if "/opt/trn_rl_repo" not in sys.path:
    sys.path.insert(0, "/opt/trn_rl_repo")

import numpy as np
import ml_dtypes

N = 100000
E = 600000
B = 64
IN = 6
H = 128
S = 16

NG = 8                    # banks and cores
NS = 12544                # nodes per core shard (98*128)
NP2 = NS // 2             # node pairs per core
NPAD = NS * NG            # 100352
NT = 98                   # node tiles per shard
NTP = 49                  # pair tiles per shard
# dst chunks per core, in node tiles (all even); small ends, big middle
TCH = (2, 4, 8, 12, 14, 16, 16, 12, 8, 4, 2)
C = len(TCH)
NDCS = tuple(t * 128 for t in TCH)          # nodes per chunk
NPCS = tuple(n // 2 for n in NDCS)          # pairs per chunk
DOFF = tuple(int(x) for x in np.concatenate([[0], np.cumsum(NDCS)]))
POFF = tuple(d // 2 for d in DOFF)
PCOL = 80                 # papt columns: 64 PA + <=16 local P
P128 = 128

_compiled = {}


def _region_layout(JWS):
    """Per-chunk packed byte region: [xs L | mask L | bidx ndp/8 | pad]."""
    RO = [0]
    for c in range(C):
        w = int(JWS[c])
        width = 2 * w + NPCS[c] // 8
        width = (width + 31) & ~31
        RO.append(RO[-1] + width)
    return RO


def _build_nc(JWS):
    import concourse.bacc as bacc
    import concourse.mybir as mybir
    from concourse import tile

    f32 = mybir.dt.float32
    bf16 = mybir.dt.bfloat16
    fp8 = mybir.dt.float8e4
    i16 = mybir.dt.int16

    RO = _region_layout(JWS)
    XMW = RO[-1]
    JWMAX = max(int(w) for w in JWS)
    NBMAX = max(NPCS)

    nc = bacc.Bacc("TRN2", target_bir_lowering=False, debug=False)

    xm = nc.declare_dram_parameter("xm", [P128, XMW], fp8, isOutput=False)
    selw = nc.declare_dram_parameter("selw", [P128, 2 * H], bf16, isOutput=False)
    papt = nc.declare_dram_parameter("papt", [P128, NTP * 2 * PCOL], fp8,
                                     isOutput=False)
    gout = nc.declare_dram_parameter("gout", [P128, PCOL], f32, isOutput=True)

    with tile.TileContext(nc) as tc:
        with (
            tc.tile_pool(name="big", bufs=1) as big,
            tc.tile_pool(name="small", bufs=1) as small,
            tc.tile_pool(name="scp", bufs=3) as scp,
            tc.tile_pool(name="d32p", bufs=2) as d32p,
            tc.tile_pool(name="hbuf", bufs=6) as hbuf,
            tc.tile_pool(name="psz", bufs=3, space="PSUM") as pszp,
            tc.tile_pool(name="psG", bufs=1, space="PSUM") as psGp,
        ):
            # preload the activation-function table while DMAs run
            warm = small.tile([1, 2], f32)
            nc.vector.memset(warm[:], 0.0)
            nc.scalar.activation(out=warm[:], in_=warm[:],
                                 func=mybir.ActivationFunctionType.Copy)

            xm_t = big.tile([P128, XMW], fp8, tag="xmb")
            dt = big.tile([P128, NP2], bf16, tag="dt")
            papt_t = big.tile([P128, NTP * 2 * PCOL], fp8, tag="papt")

            selw_t = small.tile([P128, H], bf16, name="selw_e")
            selwo_t = small.tile([P128, H], bf16, name="selw_o")

            scs = [None] * C
            d32s = [None] * C

            def dma_xm(c0, c1):
                nc.sync.dma_start(out=xm_t[:, RO[c0]:RO[c1]],
                                  in_=xm[:, RO[c0]:RO[c1]])

            def dma_papt(t0, t1):
                p0, p1 = t0 * 2 * PCOL, t1 * 2 * PCOL
                nc.sync.dma_start(out=papt_t[:, p0:p1], in_=papt[:, p0:p1])

            def scan_c(c):
                o = RO[c]
                w = int(JWS[c])
                sc = scp.tile([P128, JWMAX], f32, tag=f"sc{c % 3}",
                              name=f"scan_{c}")
                nc.vector.tensor_tensor_scan(
                    out=sc[:, 0:w],
                    data0=xm_t[:, o + w:o + 2 * w],
                    data1=xm_t[:, o:o + w],
                    initial=0.0, op0=mybir.AluOpType.mult,
                    op1=mybir.AluOpType.add,
                )
                scs[c] = sc

            def g2_c(c):
                ndp = NPCS[c]
                o = RO[c] + 2 * int(JWS[c])
                d32 = d32p.tile([P128, NBMAX], f32, tag=f"d32{c % 2}",
                                name=f"d32_{c}")
                nc.gpsimd.ap_gather(
                    out_ap=d32[:, 0:ndp], in_ap=scs[c][:, 0:int(JWS[c])],
                    idxs_ap=xm_t[:, o:o + ndp // 8].bitcast(i16),
                    channels=P128, num_elems=int(JWS[c]), d=1, num_idxs=ndp,
                )
                d32s[c] = d32

            def cvt_c(c):
                p0, ndp = POFF[c], NPCS[c]
                if c <= 4:
                    # early chunks: Act is idle while DVE is scan-bound
                    nc.scalar.activation(
                        out=dt[:, p0:p0 + ndp], in_=d32s[c][:, 0:ndp],
                        func=mybir.ActivationFunctionType.Copy,
                    )
                else:
                    nc.vector.tensor_copy(out=dt[:, p0:p0 + ndp],
                                          in_=d32s[c][:, 0:ndp])

            # ---------- issue order ----------
            dma_xm(0, 1)
            dma_xm(1, 2)
            dma_xm(2, 3)
            dma_xm(3, 4)
            nc.sync.dma_start(out=selw_t[:], in_=selw[:, 0:H])
            nc.sync.dma_start(out=selwo_t[:], in_=selw[:, H:2 * H])
            dma_xm(4, 5)
            dma_xm(5, 6)
            dma_xm(6, 7)
            dma_papt(0, 24)
            dma_xm(7, 8)
            dma_xm(8, 9)
            dma_papt(24, NTP)
            dma_xm(9, C)

            # ---------- fused pipeline: each chunk's z/relu/G batches are
            # emitted right after its cvt, so every engine's in-order queue
            # matches data-readiness order ----------
            G_ps = psGp.tile([P128, PCOL], f32, tag="G")
            QB = 4                     # pair-tiles per batch (2 mms each)
            batches_of = {}
            for c in range(C):
                t = POFF[c] // 128
                left = TCH[c] // 2
                bl = []
                while left > 0:
                    sz = min(QB, left)
                    bl.append((t, sz))
                    t += sz
                    left -= sz
                batches_of[c] = bl

            def z_mms(t0, m, ps):
                for u in range(m):
                    p0 = (t0 + u) * P128
                    for par, rw in ((0, selw_t), (1, selwo_t)):
                        nc.tensor.matmul(
                            out=ps[:, (2 * u + par) * H:(2 * u + par + 1) * H],
                            lhsT=dt[:, p0:p0 + P128],
                            rhs=rw[:],
                            start=True, stop=True,
                        )

            def g_mms(t0, m, h1):
                for u in range(m):
                    t = t0 + u
                    for par in (0, 1):
                        nc.tensor.matmul(
                            out=G_ps[:],
                            lhsT=h1[:, (2 * u + par) * H:(2 * u + par + 1) * H],
                            rhs=papt_t[:, (2 * t + par) * PCOL:
                                       (2 * t + par + 1) * PCOL],
                            start=(t == 0 and par == 0),
                            stop=(t == NTP - 1 and par == 1),
                        )

            prev = [None]

            def emit_batches(c):
                for (t0, m) in batches_of[c]:
                    ps = pszp.tile([P128, QB * 2 * H], f32, tag="z")
                    z_mms(t0, m, ps)
                    h1 = hbuf.tile([P128, QB * 2 * H], fp8, tag="h1")
                    if t0 >= POFF[6] // 128:
                        # phase A is done by these late batches; split their
                        # relus across Act and DVE to shorten the drain
                        hm = m * H
                        nc.scalar.activation(
                            out=h1[:, :hm], in_=ps[:, :hm],
                            func=mybir.ActivationFunctionType.Relu,
                        )
                        nc.vector.tensor_scalar_max(
                            out=h1[:, hm:2 * m * H], in0=ps[:, hm:2 * m * H],
                            scalar1=0.0,
                        )
                    else:
                        nc.scalar.activation(
                            out=h1[:, :2 * m * H], in_=ps[:, :2 * m * H],
                            func=mybir.ActivationFunctionType.Relu,
                        )
                    if prev[0] is not None:
                        g_mms(*prev[0])
                    prev[0] = (t0, m, h1)

            scan_c(0)
            scan_c(1)
            g2_c(0)
            for c in range(2, C):
                scan_c(c)
                g2_c(c - 1)
                cvt_c(c - 2)
                if c >= 3:
                    emit_batches(c - 3)
            g2_c(C - 1)
            cvt_c(C - 2)
            emit_batches(C - 3)
            cvt_c(C - 1)
            emit_batches(C - 2)
            emit_batches(C - 1)
            g_mms(*prev[0])

            G_sb = small.tile([P128, PCOL], f32)
            nc.vector.tensor_copy(out=G_sb[:], in_=G_ps[:])
            nc.sync.dma_start(out=gout[:], in_=G_sb[:])

    nc.compile()
    return nc


def _preprocess(x, edge_index, batch_idx):
    """Integer/structure preprocessing -> per-core device inputs."""
    src = np.asarray(edge_index[0], dtype=np.int64)
    dst = np.asarray(edge_index[1], dtype=np.int64)

    deg = (np.bincount(dst, minlength=N) + 1).astype(np.float32)
    dis = (1.0 / np.sqrt(deg)).astype(np.float32)
    sq = np.sqrt(deg).astype(np.float32)
    dis_pad = np.zeros(NPAD, np.float32)
    dis_pad[:N] = dis
    sq_pad = np.zeros(NPAD, np.float32)
    sq_pad[:N] = sq

    bi = np.asarray(batch_idx, dtype=np.int64)
    cnt = np.bincount(bi, minlength=B).astype(np.float32)

    x_np = np.asarray(x, dtype=np.float32)
    x_pad = np.zeros((NPAD, IN), np.float32)
    x_pad[:N] = x_np
    disx = x_pad * dis_pad[:, None]          # [NPAD, 6]

    # ---- pooling matrices (dense PA = P @ A) ----
    loop = np.arange(N, dtype=np.int64)
    src2 = np.concatenate([src, loop])
    dst2 = np.concatenate([dst, loop])
    w = (dis[src2] * dis[dst2]).astype(np.float64)
    flat = bi[dst2] * NPAD + src2
    PA = np.bincount(flat, weights=w, minlength=B * NPAD).reshape(B, NPAD)
    PA = PA.astype(np.float32)
    Pm = np.zeros((B, NPAD), np.float32)
    Pm[bi, np.arange(N)] = 1.0
    papt_full = (np.concatenate([PA, Pm], axis=0) * dis_pad[None, :]).T  # [NPAD,128]

    # graph span per core (for the P columns)
    first_graph = np.zeros(NG, np.int64)
    span = np.zeros(NG, np.int64)
    for k in range(NG):
        lo, hi = k * NS, min((k + 1) * NS, N)
        if lo >= N:
            first_graph[k] = B - 1
            span[k] = 1
            continue
        gset = bi[lo:hi]
        first_graph[k] = gset[0]
        span[k] = gset[-1] - gset[0] + 1
        assert span[k] <= PCOL - B, f"graph span {span[k]} > {PCOL - B}"

    # ---- per-core edge banking ----
    core = dst // NS
    dst_local = dst - core * NS
    # bank = rank within (core, dst_local) group mod NG (balances each
    # node's edges across banks)
    key0 = core * NS + dst_local
    order0 = np.argsort(key0, kind="stable")
    grp = key0[order0]
    starts = np.r_[True, grp[1:] != grp[:-1]]
    run_id = np.cumsum(starts) - 1
    run_first = np.where(starts)[0]
    rank = np.arange(E) - run_first[run_id]
    bank_e = np.empty(E, np.int64)
    # rotate the bank start per node PAIR: both nodes of a pair share the
    # rotation so their per-bank segments overlap (s = max of the two),
    # and rotations balance the banks globally
    bank_e[order0] = (rank + (dst_local[order0] // 2)) % NG

    f8 = ml_dtypes.float8_e4m3
    disx_f8 = disx.astype(f8)
    sq_f8 = sq_pad.astype(f8)

    # per-(node,bank) edge counts; +1 self edge in the slackest bank
    le = np.zeros((NPAD, NG), np.int32)
    np.add.at(le, (dst, bank_e), 1)

    JWS = [0] * C
    per_core = []
    for k in range(NG):
        n0 = k * NS
        lek = le[n0:n0 + NS].copy()            # [NS, NG]
        pv = lek.reshape(NP2, 2, NG)
        slack = pv[:, ::-1, :] - pv            # partner_count - own
        # rotate tie-breaking per node so forced +1s spread across banks
        rot = (np.arange(NG)[None, :] + np.arange(NS)[:, None]) % NG
        score = slack.reshape(NS, NG).astype(np.int64) * 16 + rot
        gstar = np.argmax(score, axis=1)
        realn = (np.arange(n0, n0 + NS) < N)
        nn = np.arange(NS)[realn]
        lek[nn, gstar[realn]] += 1
        s = np.maximum(lek.reshape(NP2, 2, NG)[:, 0, :],
                       lek.reshape(NP2, 2, NG)[:, 1, :])   # [NP2, NG]
        per_core.append((lek, gstar, s))
        for c in range(C):
            m0, m1 = POFF[c], POFF[c + 1]
            ln = 1 + s[m0:m1].sum(axis=0)
            JWS[c] = max(JWS[c], int(ln.max()))
    JWS = [((wv + 31) // 32) * 32 for wv in JWS]
    RO = _region_layout(JWS)
    XMW = RO[-1]

    # edge slot ranks: sort edges by (core, bank, dst_local)
    keyE = (core * NG + bank_e) * NS + dst_local
    orderE = np.argsort(keyE, kind="stable")
    src_s = src[orderE]
    core_s = core[orderE]
    bank_s = bank_e[orderE]
    dstl_s = dst_local[orderE]
    grpE = keyE[orderE]
    startsE = np.r_[True, grpE[1:] != grpE[:-1]]
    runE = np.cumsum(startsE) - 1
    runE_first = np.where(startsE)[0]
    rankE = np.arange(E) - runE_first[runE]

    xm_all = np.zeros((NG, P128, XMW), f8)
    feat6 = np.arange(6)
    for k in range(NG):
        lek, gstar, s = per_core[k]
        n0 = k * NS
        ek = core_s == k
        e_src = src_s[ek]
        e_bank = bank_s[ek]
        e_dstl = dstl_s[ek]
        e_rank = rankE[ek]
        e_chunk = np.searchsorted(np.asarray(DOFF[1:]), e_dstl, side="right")
        for c in range(C):
            wv = int(JWS[c])
            ndp = NPCS[c]
            m0, m1 = POFF[c], POFF[c + 1]
            b0 = RO[c]
            sc = s[m0:m1]                       # [ndp, NG]
            # pair start positions per bank (lead slot at 0)
            pos = np.zeros((ndp, NG), np.int64)
            np.cumsum(sc[:-1], axis=0, out=pos[1:])
            pos += 1
            ends = pos + sc - 1                 # [ndp, NG]

            xs_c = np.zeros((P128, wv), f8)
            mk_c = np.ones((P128, wv), f8)
            for g in range(NG):
                nzm = sc[:, g] > 0
                cols = pos[nzm, g]
                mk_c[16 * g:16 * g + 14, cols] = 0.0
                mk_c[16 * g:16 * (g + 1), 0] = 0.0

            # edge values
            ec = e_chunk == c
            if ec.any():
                g_e = e_bank[ec]
                pr = e_dstl[ec] // 2 - m0
                par = e_dstl[ec] & 1
                slot = pos[pr, g_e] + e_rank[ec]
                rows6 = (16 * g_e + 6 * par)[:, None] + feat6[None, :]
                vals = disx_f8[e_src[ec]]       # [ne, 6]
                xs_c[rows6.ravel(), np.repeat(slot, 6)] = vals.ravel()

            # self edges: node n (real) in bank gstar[n] at its last own slot
            nloc = np.arange(m0 * 2, m1 * 2)
            realn = (n0 + nloc) < N
            nl = nloc[realn]
            if len(nl):
                gsn = gstar[nl]
                pr = nl // 2 - m0
                par = nl & 1
                own_len = lek[nl, gsn]
                slot = pos[pr, gsn] + own_len - 1
                rows6 = (16 * gsn + 6 * par)[:, None] + feat6[None, :]
                vals = disx_f8[n0 + nl]
                xs_c[rows6.ravel(), np.repeat(slot, 6)] = vals.ravel()
                # sqrt(deg) lanes at the pair start of the self bank
                srow = 16 * gsn + 12 + par
                spos = pos[pr, gsn]
                xs_c[srow, spos] = sq_f8[n0 + nl]

            # pair end positions -> bidx (0 for fully-empty pairs)
            bx_c = np.zeros((P128, ndp // 16), np.int16)
            for g in range(NG):
                bvals = np.where(sc[:, g] > 0, ends[:, g], 0)
                bx_c[16 * g:16 * (g + 1)] = (
                    bvals.reshape(ndp // 16, 16).T.astype(np.int16)
                )

            xm_all[k, :, b0:b0 + wv] = xs_c
            xm_all[k, :, b0 + wv:b0 + 2 * wv] = mk_c
            xm_all[k, :, b0 + 2 * wv:b0 + 2 * wv + ndp // 8] = bx_c.view(f8)

    # papt per core, parity-interleaved per pair-tile, device layout
    papt_all = np.zeros((NG, P128, NTP * 2 * PCOL), f8)
    for k in range(NG):
        n0 = k * NS
        pk = np.zeros((NS, PCOL), np.float32)
        pk[:, :B] = papt_full[n0:n0 + NS, :B]
        b0, sp = first_graph[k], span[k]
        pk[:, B:B + sp] = papt_full[n0:n0 + NS, B + b0:B + b0 + sp]
        # node 2(t*128+p)+par -> papt_all[p, (2t+par)*PCOL + j]
        pk4 = pk.reshape(NTP, P128, 2, PCOL)        # [t, p, par, j]
        papt_all[k] = (
            pk4.transpose(1, 0, 2, 3).reshape(P128, NTP * 2 * PCOL).astype(f8)
        )

    return {
        "JW": tuple(JWS),
        "JWS": JWS,
        "xm_all": xm_all,
        "papt_all": papt_all,
        "first_graph": first_graph,
        "span": span,
        "cnt": cnt,
    }


def _head(G, cnt, inputs):
    f = np.float32
    W2 = np.asarray(inputs["W2"], f)
    b2 = np.asarray(inputs["b2"], f)
    Wg = np.asarray(inputs["Wg"], f)
    bg = np.asarray(inputs["bg"], f)
    Et = np.asarray(inputs["Et"], f)
    Ek = np.asarray(inputs["Ek"], f)
    Ev = np.asarray(inputs["Ev"], f)
    Wp = np.asarray(inputs["Wp"], f)
    bp = np.asarray(inputs["bp"], f)
    Ekid = np.asarray(inputs["Ekid"], f)
    Wc = np.asarray(inputs["Wc"], f)
    bc = np.asarray(inputs["bc"], f)
    Wl = np.asarray(inputs["Wl"], f)
    bl = np.asarray(inputs["bl"], f)
    Wm1 = np.asarray(inputs["Wm1"], f)
    bm1 = np.asarray(inputs["bm1"], f)
    Wm2 = np.asarray(inputs["Wm2"], f)
    bm2 = np.asarray(inputs["bm2"], f)
    st = np.asarray(inputs["sol_type_idx"], np.int64)
    sk = np.asarray(inputs["sol_key_idx"], np.int64)
    sv = np.asarray(inputs["sol_val_idx"], np.int64)
    kid = np.asarray(inputs["kernel_id"], np.int64)
    cond = np.asarray(inputs["cond_vec"], f)
    loc = np.asarray(inputs["local_feats"], f)

    relu = lambda a: np.maximum(a, 0.0).astype(f)

    Ph2 = G[:B] @ W2 + cnt[:, None] * b2[None, :] + G[B:]
    g = (Ph2 / np.maximum(cnt, 1.0)[:, None]) @ Wg + bg

    seq_mean = np.concatenate(
        [Et[st].mean(axis=1), Ek[sk].mean(axis=1), Ev[sv].mean(axis=1)], axis=-1
    ).astype(f)
    p = relu(seq_mean @ Wp + bp)
    kvec = Ekid[kid]
    c = relu(cond @ Wc + bc)
    l = relu(loc @ Wl + bl)
    xf = np.concatenate([g, p, kvec, c, l], axis=1).astype(f)
    return (relu(xf @ Wm1 + bm1) @ Wm2 + bm2).astype(f)


def kernel(**inputs) -> np.ndarray:
    from concourse.bass_utils import run_bass_kernel_spmd

    pre = _preprocess(inputs["x"], inputs["edge_index"], inputs["batch_idx"])
    sig = pre["JW"]
    if sig not in _compiled:
        _compiled[sig] = _build_nc(tuple(pre["JWS"]))
    nc = _compiled[sig]

    W1 = np.asarray(inputs["W1"], np.float32)
    b1 = np.asarray(inputs["b1"], np.float32)
    W1b = W1.astype(ml_dtypes.bfloat16)
    b1b = b1.astype(ml_dtypes.bfloat16)
    selw = np.zeros((P128, 2 * H), ml_dtypes.bfloat16)
    for g in range(NG):
        selw[16 * g:16 * g + 6, 0:H] = W1b            # even lanes -> z_e
        selw[16 * g + 6:16 * g + 12, H:2 * H] = W1b   # odd lanes -> z_o
        selw[16 * g + 12, 0:H] = b1b                  # even sqrt(deg) lane
        selw[16 * g + 13, H:2 * H] = b1b              # odd sqrt(deg) lane

    in_maps = []
    for k in range(NG):
        in_maps.append({
            "xm": pre["xm_all"][k],
            "selw": selw,
            "papt": pre["papt_all"][k],
        })

    res = run_bass_kernel_spmd(nc, in_maps, core_ids=list(range(NG)))

    Gpa = np.zeros((B, H), np.float64)
    Gp = np.zeros((B, H), np.float64)
    for k, r in enumerate(res.results):
        gt = r["gout"].astype(np.float64)      # [128 f, 80 c]
        Gpa += gt[:, :B].T
        b0, sp = pre["first_graph"][k], pre["span"][k]
        Gp[b0:b0 + sp] += gt[:, B:B + sp].T
    G = np.concatenate([Gpa, Gp], axis=0).astype(np.float32)   # [128, H]

    return _head(G, pre["cnt"], inputs)
